# revision 13
# baseline (speedup 1.0000x reference)
"""Bass/Tile MHA kernel for trn2 — builder + host shard/unshard helpers.

Per-core work (8 cores): core c handles batch b=c//2, head-group g=c%2
(8 of 16 heads).

All matmuls are full 128x128-tile instructions with M=128: measured PE
cost is ~(output free size) * 0.44ns regardless of M/K, so merging the
64-row head-pair splits halves matmul count vs the hh/mh-split version.
All-bf16 numerics: fp8 P/V (DoubleRow) was measured at 2.9% max rel err
— e4m3's ~3% quantization noise passes straight through the softmax
average (signal and noise shrink together), so it cannot meet the 2e-2
gate.

The engine-side work (exp of 33.5M scores + PSUM->SBUF copies) is ~half
of the runtime budget per engine; it is split between ScalarE (true exp)
and VectorE (Schraudolph fast-exp) by per-slot assignment, with GpSimd
carrying only the partition broadcasts (it cannot read PSUM).

Dataflow (bf16 matmuls, fp32 PSUM accumulate):
  qkT[d_h, t]  = w_qk^T x           (lhsT=w_qk tile, rhs=x^T tile)
  V[t, d_v]    = x w_v              (lhsT=x^T tile [128 tok], rhs=w_v)
  S^T[k, q]    = (K^T)^T Q^T        (K=128 zero-padded contraction, M=128)
  P^T          = exp(S^T)           (engine split tunable per (kt,hh) slot)
  av[:, q]     = Σ_h (V'_h|0)^T P_h^T  (per-head zero-padded V tiles, both
                                     heads accumulate one PSUM bank; ones
                                     column gives denoms at rows 0 / 64)
  outT         = av * bcast(1/l)
  y[t, j]      = outT^T w_out       (+ b_out and cross-core sum on host)
"""

import math

import numpy as np
import ml_dtypes

import concourse.bass as bass
import concourse.mybir as mybir
import concourse.tile as tile
from concourse import bacc

F32 = mybir.dt.float32
BF16 = mybir.dt.bfloat16
I16 = mybir.dt.int16
AF = mybir.ActivationFunctionType
OP = mybir.AluOpType

DIM = 768
PH = 48
NP = 4          # head pairs per core
HC = 8          # heads per core
NDT = DIM // 128  # 6 contraction tiles for the projections

# Schraudolph fast-exp in bf16 bit space: bits = round(x*128/ln2 + (127*128 - C))
SCH_A = 128.0 / math.log(2.0)
SCH_C = 4.7
# +0.5: the fp32->int16 convert truncates, this re-centers it to round-nearest
SCH_B = 127.0 * 128.0 - SCH_C + 0.5


def build_kernel(T=2048,
                 dve_slots=frozenset({0, 2, 4, 5, 6, 8, 10, 12, 14}),
                 num_devices=8):
    """Returns compiled Bacc module. dve_slots: which of 16 (kt*2+hh)%16
    pipeline slots run fast-exp on VectorE instead of exp on ScalarE."""
    KT = T // 128                 # k-tiles (token tiles)
    QCW = min(512, T)             # q chunk width (one PSUM bank)
    NQG = T // QCW                # q groups, one chunk each

    nc = bacc.Bacc("TRN2", target_bir_lowering=False, debug=False,
                   num_devices=num_devices)

    xt_d = nc.dram_tensor("xt", (DIM, T), BF16, kind="ExternalInput")
    wqk_d = nc.dram_tensor("wqk", (DIM, NP * 2 * 128), BF16, kind="ExternalInput")
    wv_d = nc.dram_tensor("wv", (DIM, HC * PH), BF16, kind="ExternalInput")
    wo_d = nc.dram_tensor("wo", (NP * 128, DIM), BF16, kind="ExternalInput")
    bqk_d = nc.dram_tensor("bqk", (128, NP * 2), F32, kind="ExternalInput")
    y_d = nc.dram_tensor("y", (T, DIM), F32, kind="ExternalOutput")

    with tile.TileContext(nc) as tc:
        with (
            tc.tile_pool(name="const", bufs=1) as cpool,
            tc.tile_pool(name="pt", bufs=8) as ptpool,
            tc.tile_pool(name="norm", bufs=2) as npool,
            tc.tile_pool(name="ysb", bufs=2) as ypool,
            tc.tile_pool(name="st", bufs=5, space="PSUM") as stpool,
            tc.tile_pool(name="av", bufs=2, space="PSUM") as avpool,
            tc.tile_pool(name="pj", bufs=1, space="PSUM") as pjpool,
        ):
            # ---- persistent SBUF tensors; per-dmodel-tile tensors are
            # separate tiles so the first projection matmuls only wait for
            # their own DMA, not the whole array ----
            xt_sb = [cpool.tile([128, T], BF16, tag=f"xt{i}", name=f"xt{i}")
                     for i in range(NDT)]
            wqk_sb = [cpool.tile([128, NP * 2 * 128], BF16, tag=f"wqk{i}",
                                 name=f"wqk{i}") for i in range(NDT)]
            wv_sb = [cpool.tile([128, HC * PH], BF16, tag=f"wv{i}",
                                name=f"wv{i}") for i in range(NDT)]
            wo_sb = cpool.tile([128, NP, DIM], BF16, tag="wo")
            bqk_sb = cpool.tile([128, NP * 2], F32, tag="bqk")
            # per-head Q^T/K^T, rows 48-127 zeroed: padding the QK
            # contraction to K=128 keeps every matmul in 128x128 mode
            qk_sb = cpool.tile([128, HC, 2, T], BF16, tag="qk")
            # V' tiles per (kt, pair, hh): hh=0: col 0 = ones (softmax denom
            # on PSUM row 0), 1-48 = V, 49-127 zero; hh=1: 0-63 zero, 64 =
            # ones (denom row 64), 65-112 = V, 113-127 zero. Zero halves let
            # both heads of a pair accumulate into one shared PSUM bank with
            # full M=128 instructions.
            v_sb = cpool.tile([128, KT, NP, 2, 128], BF16, tag="v")
            outT_sb = cpool.tile([128, NP, T], BF16, tag="outT")

            # ---- input DMAs (dt=0 tensors first: qkT_proj(0) starts after
            # the first three transfers) ----
            for dt_i in range(NDT):
                nc.sync.dma_start(wqk_sb[dt_i][:], wqk_d[dt_i * 128:(dt_i + 1) * 128, :])
                nc.sync.dma_start(xt_sb[dt_i][:], xt_d[dt_i * 128:(dt_i + 1) * 128, :])
                nc.sync.dma_start(wv_sb[dt_i][:], wv_d[dt_i * 128:(dt_i + 1) * 128, :])
            nc.sync.dma_start(bqk_sb[:], bqk_d[:])
            for p in range(NP):
                nc.sync.dma_start(wo_sb[:, p, :], wo_d[p * 128:(p + 1) * 128, :])

            # ones columns for the softmax-denominator trick; zero pads
            nc.gpsimd.memset(v_sb[:, :, :, 0, 0:1], 1.0)
            nc.gpsimd.memset(v_sb[:, :, :, 0, PH + 1:128], 0.0)
            nc.gpsimd.memset(v_sb[:, :, :, 1, 0:64], 0.0)
            nc.gpsimd.memset(v_sb[:, :, :, 1, 64:65], 1.0)
            nc.gpsimd.memset(v_sb[:, :, :, 1, 64 + PH + 1:128], 0.0)
            nc.gpsimd.memset(qk_sb[32:64, :, :, :], 0.0)
            nc.gpsimd.memset(qk_sb[64:96, :, :, :], 0.0)
            nc.gpsimd.memset(qk_sb[96:128, :, :, :], 0.0)
            # pad rows (49-63, 113-127) must be finite; 32-aligned starts,
            # the real rows 32-47 / 96-111 are overwritten by normalize
            nc.gpsimd.memset(outT_sb[32:64, :, :], 0.0)
            nc.gpsimd.memset(outT_sb[96:128, :, :], 0.0)

            def qkT_proj(p):
                # qkT[d_h, t] for pair p: rows 0-47 head A dims, 64-111 head B
                for qk in range(2):
                    col0 = (p * 2 + qk) * 128
                    for tcI in range(T // QCW):
                        ps = pjpool.tile([128, QCW], F32, tag="pj")
                        for dt_i in range(NDT):
                            nc.tensor.matmul(
                                ps[:, :],
                                wqk_sb[dt_i][:, col0:col0 + 128],
                                xt_sb[dt_i][:, tcI * QCW:(tcI + 1) * QCW],
                                start=(dt_i == 0), stop=(dt_i == NDT - 1),
                                skip_group_check=True)
                        tsl = np.s_[tcI * QCW:(tcI + 1) * QCW]
                        nc.scalar.activation(
                            qk_sb[0:PH, p * 2, qk, tsl], ps[0:PH, :],
                            AF.Identity, bias=bqk_sb[0:PH, p * 2 + qk:p * 2 + qk + 1])
                        nc.scalar.activation(
                            qk_sb[0:PH, p * 2 + 1, qk, tsl], ps[64:64 + PH, :],
                            AF.Identity, bias=bqk_sb[64:64 + PH, p * 2 + qk:p * 2 + qk + 1])

            def v_proj():
                for tt in range(KT):
                    # full-bank tile so every pj slot stays bank-aligned
                    psb = pjpool.tile([128, 512], F32, tag="pj")
                    ps = psb[:, 0:HC * PH]
                    for dt_i in range(NDT):
                        nc.tensor.matmul(
                            ps[:, :],
                            xt_sb[dt_i][:, tt * 128:(tt + 1) * 128],
                            wv_sb[dt_i][:],
                            start=(dt_i == 0), stop=(dt_i == NDT - 1),
                            skip_group_check=True)
                    ps4 = ps.rearrange("p (pp two d) -> p pp two d", two=2, d=PH)
                    nc.vector.tensor_copy(
                        v_sb[:, tt, :, 0, 1:PH + 1], ps4[:, :, 0, :])
                    nc.vector.tensor_copy(
                        v_sb[:, tt, :, 1, 65:65 + PH], ps4[:, :, 1, :])

            def attention(p, qg):
                cs = np.s_[qg * QCW:(qg + 1) * QCW]
                av = avpool.tile([128, QCW], F32, tag="av", name="av")
                LEAD = 3  # AV trails QK by 3 k-tiles so exp latency hides
                pts = {}

                def qk_emit(kt):
                    for hh in range(2):
                        st = stpool.tile([128, QCW], F32, tag="st")
                        nc.tensor.matmul(
                            st[:, :],
                            qk_sb[:, p * 2 + hh, 1, kt * 128:(kt + 1) * 128],
                            qk_sb[:, p * 2 + hh, 0, cs],
                            start=True, stop=True,
                            skip_group_check=True)
                        pt = ptpool.tile([128, QCW], BF16, tag="pt")
                        if (kt * 2 + hh) % 16 in dve_slots:
                            nc.vector.tensor_scalar(
                                pt[:].bitcast(I16), st[:], SCH_A, SCH_B,
                                OP.mult, OP.add)
                        else:
                            nc.scalar.activation(pt[:], st[:], AF.Exp)
                        pts[(kt, hh)] = pt

                def av_emit(kt):
                    for hh in range(2):
                        nc.tensor.matmul(
                            av[:, :],
                            v_sb[:, kt, p, hh, :],
                            pts.pop((kt, hh))[:],
                            start=(kt == 0 and hh == 0),
                            stop=(kt == KT - 1 and hh == 1),
                            skip_group_check=True)

                for kt in range(KT + LEAD):
                    if kt < KT:
                        qk_emit(kt)
                    if kt >= LEAD:
                        av_emit(kt - LEAD)
                # normalize + bias into outT (denominators live in rows 0/64).
                # partition_broadcast is only reliable with base-0 in/out APs,
                # so each head gets its own base-0 recip + broadcast tiles.
                r2a = npool.tile([128, QCW], F32, tag="r2", name="r2a")
                r2b = npool.tile([128, QCW], F32, tag="r2", name="r2b")
                rbca = npool.tile([128, QCW], F32, tag="rbc", name="rbca")
                rbcb = npool.tile([128, QCW], F32, tag="rbc", name="rbcb")
                lra = npool.tile([128, QCW], F32, tag="lr", name="lra")
                lrb = npool.tile([128, QCW], F32, tag="lr", name="lrb")
                nc.scalar.copy(lra[0:1, :], av[0:1, :])
                nc.scalar.copy(lrb[0:1, :], av[64:65, :])
                nc.vector.reciprocal_approx_fast(r2a[0:1, :], lra[0:1, :])
                nc.vector.reciprocal_approx_fast(r2b[0:1, :], lrb[0:1, :])
                nc.gpsimd.partition_broadcast(rbca[0:PH + 1, :], r2a[0:1, :])
                nc.gpsimd.partition_broadcast(rbcb[0:PH + 1, :], r2b[0:1, :])
                nc.vector.tensor_mul(outT_sb[0:PH + 1, p, cs],
                                     av[0:PH + 1, :], rbca[0:PH + 1, :])
                nc.vector.tensor_mul(outT_sb[64:64 + PH + 1, p, cs],
                                     av[64:64 + PH + 1, :], rbcb[0:PH + 1, :])

            def final_proj(qg):
                for tt in range(QCW // 128):
                    t0 = qg * QCW + tt * 128
                    ysb = ypool.tile([128, DIM], F32, tag="ysb")
                    for jc in range(2):
                        js = np.s_[jc * 384:(jc + 1) * 384]
                        psb = pjpool.tile([128, 512], F32, tag="pj", name=f"yp{jc}")
                        ps = psb[:, 0:384]
                        for p in range(NP):
                            nc.tensor.matmul(
                                ps[:, :],
                                outT_sb[:, p, t0:t0 + 128],
                                wo_sb[:, p, js],
                                start=(p == 0), stop=(p == NP - 1),
                                skip_group_check=True)
                        if (tt + jc) % 2 == 0:
                            nc.scalar.copy(ysb[:, js], ps[:])
                        else:
                            nc.vector.tensor_copy(ysb[:, js], ps[:])
                    nc.sync.dma_start(y_d[t0:t0 + 128, :], ysb[:])

            # ---- emission order (scheduling priority) ----
            qkT_proj(0)
            v_proj()
            for qg in range(NQG):
                for p in range(NP):
                    if qg == 0 and p + 1 < NP:
                        qkT_proj(p + 1)
                    attention(p, qg)
                final_proj(qg)

    nc.compile()
    return nc


# ---------------- host-side sharding ----------------

def host_prep(x, w_in, b_in, w_out, T=2048):
    """Full inputs -> list of 8 per-core input dicts."""
    scale = 1.0 / math.sqrt(PH)
    wr = np.asarray(w_in).reshape(DIM, 16, 3, PH)
    br = np.asarray(b_in).reshape(16, 3, PH)
    wog = np.asarray(w_out)  # (768, 768), row dv = h*48+d
    in_maps = []
    for c in range(8):
        b, g = divmod(c, 2)
        wqk = np.zeros((DIM, NP * 2 * 128), np.float32)
        bqk = np.zeros((128, NP * 2), np.float32)
        wv = np.zeros((DIM, HC * PH), np.float32)
        wo = np.zeros((NP * 128, DIM), np.float32)
        for p in range(NP):
            for hh, base in ((0, 0), (1, 64)):
                gh = g * 8 + p * 2 + hh
                wqk[:, (p * 2) * 128 + base:(p * 2) * 128 + base + PH] = wr[:, gh, 0] * scale
                wqk[:, (p * 2 + 1) * 128 + base:(p * 2 + 1) * 128 + base + PH] = wr[:, gh, 1]
                bqk[base:base + PH, p * 2] = br[gh, 0] * scale
                bqk[base:base + PH, p * 2 + 1] = br[gh, 1]
                wv[:, (p * 2 + hh) * PH:(p * 2 + hh + 1) * PH] = wr[:, gh, 2]
                wo[p * 128 + base + 1:p * 128 + base + 1 + PH, :] = wog[gh * PH:(gh + 1) * PH, :]
        in_maps.append({
            "xt": np.ascontiguousarray(np.asarray(x)[b].T).astype(ml_dtypes.bfloat16),
            "wqk": wqk.astype(ml_dtypes.bfloat16),
            "wv": wv.astype(ml_dtypes.bfloat16),
            "wo": wo.astype(ml_dtypes.bfloat16),
            "bqk": bqk,
        })
    return in_maps


def host_post(results, b_out, b_in, w_out, B=4, T=2048):
    # the V bias contributes bv @ w_out, a per-column constant: add on host
    bv_all = np.asarray(b_in).reshape(16, 3, PH)[:, 2, :].reshape(DIM)
    const = np.asarray(b_out) + bv_all @ np.asarray(w_out)
    out = np.empty((B, T, DIM), np.float32)
    for b in range(B):
        out[b] = results[2 * b]["y"] + results[2 * b + 1]["y"] + const[None, :]
    return out


# ---------------- self-contained kernel() entry point ----------------

_CACHED = {}


def _get_nc():
    if "nc" not in _CACHED:
        _CACHED["nc"] = build_kernel(T=2048, num_devices=8)
    return _CACHED["nc"]


def kernel(x, w_in, b_in, w_out, b_out):
    """Full-input MHA forward on 8 NeuronCores.

    x: (4, 2048, 768) f32; w_in: (768, 2304); b_in: (2304,);
    w_out: (768, 768); b_out: (768,). Returns (4, 2048, 768) f32.
    """
    from concourse.bass_utils import run_bass_kernel_spmd

    x = np.asarray(x, np.float32)
    w_in = np.asarray(w_in, np.float32)
    b_in = np.asarray(b_in, np.float32)
    w_out = np.asarray(w_out, np.float32)
    b_out = np.asarray(b_out, np.float32)

    nc = _get_nc()
    in_maps = host_prep(x, w_in, b_in, w_out, T=2048)
    res = run_bass_kernel_spmd(nc, in_maps, core_ids=list(range(8)))
    return host_post(res.results, b_out, b_in, w_out, B=4, T=2048)


# revision 16
# speedup vs baseline: 1.0910x; 1.0910x over previous
"""Bass/Tile MHA kernel for trn2 — builder + host shard/unshard helpers.

Per-core work (8 cores): core c handles batch b=c//2, head-group g=c%2
(8 of 16 heads).

All matmuls are full 128x128-tile instructions with M=128: measured PE
cost is ~(output free size) * 0.44ns regardless of M/K, so merging the
64-row head-pair splits halves matmul count vs the hh/mh-split version.
All-bf16 numerics: fp8 P/V (DoubleRow) was measured at 2.9% max rel err
— e4m3's ~3% quantization noise passes straight through the softmax
average (signal and noise shrink together), so it cannot meet the 2e-2
gate.

The engine-side work (exp of 33.5M scores + PSUM->SBUF copies) is ~half
of the runtime budget per engine; it is split between ScalarE (true exp)
and VectorE (Schraudolph fast-exp) by per-slot assignment, with GpSimd
carrying only the partition broadcasts (it cannot read PSUM).

Dataflow (bf16 matmuls, fp32 PSUM accumulate):
  qkT[d_h, t]  = w_qk^T x           (lhsT=w_qk tile, rhs=x^T tile)
  V[t, d_v]    = x w_v              (lhsT=x^T tile [128 tok], rhs=w_v)
  S^T[k, q]    = (K^T)^T Q^T        (K=128 zero-padded contraction, M=128)
  P^T          = exp(S^T)           (engine split tunable per (kt,hh) slot)
  av[:, q]     = Σ_h (V'_h|0)^T P_h^T  (per-head zero-padded V tiles, both
                                     heads accumulate one PSUM bank; ones
                                     column gives denoms at rows 0 / 64)
  outT         = av * bcast(1/l)
  y[t, j]      = outT^T w_out       (+ b_out and cross-core sum on host)
"""

import math

import numpy as np
import ml_dtypes

import concourse.bass as bass
import concourse.mybir as mybir
import concourse.tile as tile
from concourse import bacc

F32 = mybir.dt.float32
BF16 = mybir.dt.bfloat16
I16 = mybir.dt.int16
AF = mybir.ActivationFunctionType
OP = mybir.AluOpType

DIM = 768
PH = 48
NP = 4          # head pairs per core
HC = 8          # heads per core
NDT = DIM // 128  # 6 contraction tiles for the projections

# Schraudolph fast-exp in bf16 bit space: bits = round(x*128/ln2 + (127*128 - C))
SCH_A = 128.0 / math.log(2.0)
SCH_C = 4.7
# +0.5: the fp32->int16 convert truncates, this re-centers it to round-nearest
SCH_B = 127.0 * 128.0 - SCH_C + 0.5


def build_kernel(T=2048,
                 dve_slots=frozenset({0, 2, 4, 6, 8, 10, 12, 14}),
                 num_devices=8):
    """Returns compiled Bacc module. dve_slots: which of 16 (kt*2+hh)%16
    pipeline slots run fast-exp on VectorE instead of exp on ScalarE."""
    KT = T // 128                 # k-tiles (token tiles)
    QCW = min(512, T)             # q chunk width (one PSUM bank)
    NQG = T // QCW                # q groups, one chunk each

    nc = bacc.Bacc("TRN2", target_bir_lowering=False, debug=False,
                   num_devices=num_devices)

    xt_d = nc.dram_tensor("xt", (DIM, T), BF16, kind="ExternalInput")
    wqk_d = nc.dram_tensor("wqk", (DIM, NP * 2 * 128), BF16, kind="ExternalInput")
    wv_d = nc.dram_tensor("wv", (DIM, HC * PH), BF16, kind="ExternalInput")
    wo_d = nc.dram_tensor("wo", (NP * 128, DIM), BF16, kind="ExternalInput")
    bqk_d = nc.dram_tensor("bqk", (128, NP * 2), F32, kind="ExternalInput")
    y_d = nc.dram_tensor("y", (T, DIM), F32, kind="ExternalOutput")

    with tile.TileContext(nc) as tc:
        with (
            tc.tile_pool(name="const", bufs=1) as cpool,
            tc.tile_pool(name="pt", bufs=12) as ptpool,
            tc.tile_pool(name="norm", bufs=2) as npool,
            tc.tile_pool(name="ysb", bufs=2) as ypool,
            tc.tile_pool(name="st", bufs=4, space="PSUM") as stpool,
            tc.tile_pool(name="av", bufs=2, space="PSUM") as avpool,
            tc.tile_pool(name="pj", bufs=2, space="PSUM") as pjpool,
        ):
            # ---- persistent SBUF tensors; per-dmodel-tile tensors are
            # separate tiles so the first projection matmuls only wait for
            # their own DMA, not the whole array ----
            xt_sb = [cpool.tile([128, T], BF16, tag=f"xt{i}", name=f"xt{i}")
                     for i in range(NDT)]
            wqk_sb = [cpool.tile([128, NP * 2 * 128], BF16, tag=f"wqk{i}",
                                 name=f"wqk{i}") for i in range(NDT)]
            wv_sb = [cpool.tile([128, HC * PH], BF16, tag=f"wv{i}",
                                name=f"wv{i}") for i in range(NDT)]
            wo_sb = cpool.tile([128, NP, DIM], BF16, tag="wo")
            bqk_sb = cpool.tile([128, NP * 2], F32, tag="bqk")
            # per-head Q^T/K^T, rows 48-127 zeroed: padding the QK
            # contraction to K=128 keeps every matmul in 128x128 mode
            qk_sb = cpool.tile([128, HC, 2, T], BF16, tag="qk")
            # V' tiles per (kt, pair, hh): hh=0: col 0 = ones (softmax denom
            # on PSUM row 0), 1-48 = V, 49-127 zero; hh=1: 0-63 zero, 64 =
            # ones (denom row 64), 65-112 = V, 113-127 zero. Zero halves let
            # both heads of a pair accumulate into one shared PSUM bank with
            # full M=128 instructions.
            v_sb = cpool.tile([128, KT, NP, 2, 128], BF16, tag="v")
            outT_sb = cpool.tile([128, NP, T], BF16, tag="outT")

            # ---- input DMAs (dt=0 tensors first: qkT_proj(0) starts after
            # the first three transfers) ----
            for dt_i in range(NDT):
                nc.sync.dma_start(wqk_sb[dt_i][:], wqk_d[dt_i * 128:(dt_i + 1) * 128, :])
                nc.sync.dma_start(xt_sb[dt_i][:], xt_d[dt_i * 128:(dt_i + 1) * 128, :])
                nc.sync.dma_start(wv_sb[dt_i][:], wv_d[dt_i * 128:(dt_i + 1) * 128, :])
            nc.sync.dma_start(bqk_sb[:], bqk_d[:])
            for p in range(NP):
                nc.sync.dma_start(wo_sb[:, p, :], wo_d[p * 128:(p + 1) * 128, :])

            # ones columns for the softmax-denominator trick; zero pads
            nc.gpsimd.memset(v_sb[:, :, :, 0, 0:1], 1.0)
            nc.gpsimd.memset(v_sb[:, :, :, 0, PH + 1:128], 0.0)
            nc.gpsimd.memset(v_sb[:, :, :, 1, 0:64], 0.0)
            nc.gpsimd.memset(v_sb[:, :, :, 1, 64:65], 1.0)
            nc.gpsimd.memset(v_sb[:, :, :, 1, 64 + PH + 1:128], 0.0)
            nc.gpsimd.memset(qk_sb[32:64, :, :, :], 0.0)
            nc.gpsimd.memset(qk_sb[64:96, :, :, :], 0.0)
            nc.gpsimd.memset(qk_sb[96:128, :, :, :], 0.0)
            # pad rows (49-63, 113-127) must be finite; 32-aligned starts,
            # the real rows 32-47 / 96-111 are overwritten by normalize
            nc.gpsimd.memset(outT_sb[32:64, :, :], 0.0)
            nc.gpsimd.memset(outT_sb[96:128, :, :], 0.0)

            def qkT_chunks(p):
                # qkT[d_h, t] for pair p: rows 0-47 head A dims, 64-111 head B
                # as one closure per (qk, tcI) so emission can interleave with
                # attention and the PSUM->SBUF copies don't bunch up in the
                # ScalarE queue ahead of the exp ops
                def chunk(qk, tcI):
                    col0 = (p * 2 + qk) * 128
                    if True:
                        ps = pjpool.tile([128, QCW], F32, tag="pj")
                        for dt_i in range(NDT):
                            nc.tensor.matmul(
                                ps[:, :],
                                wqk_sb[dt_i][:, col0:col0 + 128],
                                xt_sb[dt_i][:, tcI * QCW:(tcI + 1) * QCW],
                                start=(dt_i == 0), stop=(dt_i == NDT - 1),
                                skip_group_check=True)

                        tsl = np.s_[tcI * QCW:(tcI + 1) * QCW]
                        nc.scalar.activation(
                            qk_sb[0:PH, p * 2, qk, tsl], ps[0:PH, :],
                            AF.Identity, bias=bqk_sb[0:PH, p * 2 + qk:p * 2 + qk + 1])
                        nc.scalar.activation(
                            qk_sb[0:PH, p * 2 + 1, qk, tsl], ps[64:64 + PH, :],
                            AF.Identity, bias=bqk_sb[64:64 + PH, p * 2 + qk:p * 2 + qk + 1])
                return [(lambda qk=qk, tcI=tcI: chunk(qk, tcI))
                        for qk in range(2) for tcI in range(T // QCW)]

            def v_proj():
                for tt in range(KT):
                    # full-bank tile so every pj slot stays bank-aligned
                    psb = pjpool.tile([128, 512], F32, tag="pj")
                    ps = psb[:, 0:HC * PH]
                    for dt_i in range(NDT):
                        nc.tensor.matmul(
                            ps[:, :],
                            xt_sb[dt_i][:, tt * 128:(tt + 1) * 128],
                            wv_sb[dt_i][:],
                            start=(dt_i == 0), stop=(dt_i == NDT - 1),
                            skip_group_check=True)
                    ps4 = ps.rearrange("p (pp two d) -> p pp two d", two=2, d=PH)
                    nc.vector.tensor_copy(
                        v_sb[:, tt, :, 0, 1:PH + 1], ps4[:, :, 0, :])
                    nc.vector.tensor_copy(
                        v_sb[:, tt, :, 1, 65:65 + PH], ps4[:, :, 1, :])

            def attention(p, qg, pref=()):
                cs = np.s_[qg * QCW:(qg + 1) * QCW]
                av = avpool.tile([128, QCW], F32, tag="av", name="av")
                LEAD = 2  # AV trails QK by 2 k-tiles so exp latency hides
                pts = {}

                def qk_emit(kt):
                    for hh in range(2):
                        st = stpool.tile([128, QCW], F32, tag="st")
                        nc.tensor.matmul(
                            st[:, :],
                            qk_sb[:, p * 2 + hh, 1, kt * 128:(kt + 1) * 128],
                            qk_sb[:, p * 2 + hh, 0, cs],
                            start=True, stop=True,
                            skip_group_check=True)
                        pt = ptpool.tile([128, QCW], BF16, tag="pt")
                        if (kt * 2 + hh) % 16 in dve_slots:
                            nc.vector.tensor_scalar(
                                pt[:].bitcast(I16), st[:], SCH_A, SCH_B,
                                OP.mult, OP.add)
                        else:
                            nc.scalar.activation(pt[:], st[:], AF.Exp)
                        pts[(kt, hh)] = pt

                def av_emit(kt):
                    for hh in range(2):
                        nc.tensor.matmul(
                            av[:, :],
                            v_sb[:, kt, p, hh, :],
                            pts.pop((kt, hh))[:],
                            start=(kt == 0 and hh == 0),
                            stop=(kt == KT - 1 and hh == 1),
                            skip_group_check=True)

                for kt in range(KT + LEAD):
                    # spread prefetched projection chunks through the block so
                    # their engine ops don't bunch ahead of the exp queue
                    if kt % 2 == 0 and kt // 2 < len(pref):
                        pref[kt // 2]()
                    if kt < KT:
                        qk_emit(kt)
                    if kt >= LEAD:
                        av_emit(kt - LEAD)
                # normalize + bias into outT (denominators live in rows 0/64).
                # partition_broadcast is only reliable with base-0 in/out APs,
                # so each head gets its own base-0 recip + broadcast tiles.
                r2a = npool.tile([128, QCW], F32, tag="r2", name="r2a")
                r2b = npool.tile([128, QCW], F32, tag="r2", name="r2b")
                rbca = npool.tile([128, QCW], F32, tag="rbc", name="rbca")
                rbcb = npool.tile([128, QCW], F32, tag="rbc", name="rbcb")
                lra = npool.tile([128, QCW], F32, tag="lr", name="lra")
                lrb = npool.tile([128, QCW], F32, tag="lr", name="lrb")
                # the recip must NOT read PSUM directly (measured 2.3% rel
                # err when it did); copy the denom rows to base-0 SBUF first
                nc.scalar.copy(lra[0:1, :], av[0:1, :])
                nc.scalar.copy(lrb[0:1, :], av[64:65, :])
                nc.vector.reciprocal_approx_fast(r2a[0:1, :], lra[0:1, :])
                nc.vector.reciprocal_approx_fast(r2b[0:1, :], lrb[0:1, :])
                nc.gpsimd.partition_broadcast(rbca[0:PH + 1, :], r2a[0:1, :])
                nc.gpsimd.partition_broadcast(rbcb[0:PH + 1, :], r2b[0:1, :])
                nc.vector.tensor_mul(outT_sb[0:PH + 1, p, cs],
                                     av[0:PH + 1, :], rbca[0:PH + 1, :])
                nc.vector.tensor_mul(outT_sb[64:64 + PH + 1, p, cs],
                                     av[64:64 + PH + 1, :], rbcb[0:PH + 1, :])

            def final_chunks(qg):
                def chunk(tt):
                    t0 = qg * QCW + tt * 128
                    ysb = ypool.tile([128, DIM], F32, tag="ysb")
                    for jc in range(2):
                        js = np.s_[jc * 384:(jc + 1) * 384]
                        psb = pjpool.tile([128, 512], F32, tag="pj", name=f"yp{jc}")
                        ps = psb[:, 0:384]
                        for p in range(NP):
                            nc.tensor.matmul(
                                ps[:, :],
                                outT_sb[:, p, t0:t0 + 128],
                                wo_sb[:, p, js],
                                start=(p == 0), stop=(p == NP - 1),
                                skip_group_check=True)
                        if (tt + jc) % 2 == 0:
                            nc.scalar.copy(ysb[:, js], ps[:])
                        else:
                            nc.vector.tensor_copy(ysb[:, js], ps[:])
                    nc.sync.dma_start(y_d[t0:t0 + 128, :], ysb[:])
                return [(lambda tt=tt: chunk(tt)) for tt in range(QCW // 128)]

            # ---- emission order (scheduling priority) ----
            for fn in qkT_chunks(0):
                fn()
            v_proj()
            for qg in range(NQG):
                for p in range(NP):
                    pref = ()
                    if qg == 0 and p + 1 < NP:
                        pref = qkT_chunks(p + 1)
                    elif p == 0 and qg > 0:
                        pref = final_chunks(qg - 1)
                    attention(p, qg, pref)
            for fn in final_chunks(NQG - 1):
                fn()

    nc.compile()
    return nc


# ---------------- host-side sharding ----------------

def host_prep(x, w_in, b_in, w_out, T=2048):
    """Full inputs -> list of 8 per-core input dicts."""
    scale = 1.0 / math.sqrt(PH)
    wr = np.asarray(w_in).reshape(DIM, 16, 3, PH)
    br = np.asarray(b_in).reshape(16, 3, PH)
    wog = np.asarray(w_out)  # (768, 768), row dv = h*48+d
    in_maps = []
    for c in range(8):
        b, g = divmod(c, 2)
        wqk = np.zeros((DIM, NP * 2 * 128), np.float32)
        bqk = np.zeros((128, NP * 2), np.float32)
        wv = np.zeros((DIM, HC * PH), np.float32)
        wo = np.zeros((NP * 128, DIM), np.float32)
        for p in range(NP):
            for hh, base in ((0, 0), (1, 64)):
                gh = g * 8 + p * 2 + hh
                wqk[:, (p * 2) * 128 + base:(p * 2) * 128 + base + PH] = wr[:, gh, 0] * scale
                wqk[:, (p * 2 + 1) * 128 + base:(p * 2 + 1) * 128 + base + PH] = wr[:, gh, 1]
                bqk[base:base + PH, p * 2] = br[gh, 0] * scale
                bqk[base:base + PH, p * 2 + 1] = br[gh, 1]
                wv[:, (p * 2 + hh) * PH:(p * 2 + hh + 1) * PH] = wr[:, gh, 2]
                wo[p * 128 + base + 1:p * 128 + base + 1 + PH, :] = wog[gh * PH:(gh + 1) * PH, :]
        in_maps.append({
            "xt": np.ascontiguousarray(np.asarray(x)[b].T).astype(ml_dtypes.bfloat16),
            "wqk": wqk.astype(ml_dtypes.bfloat16),
            "wv": wv.astype(ml_dtypes.bfloat16),
            "wo": wo.astype(ml_dtypes.bfloat16),
            "bqk": bqk,
        })
    return in_maps


def host_post(results, b_out, b_in, w_out, B=4, T=2048):
    # the V bias contributes bv @ w_out, a per-column constant: add on host
    bv_all = np.asarray(b_in).reshape(16, 3, PH)[:, 2, :].reshape(DIM)
    const = np.asarray(b_out) + bv_all @ np.asarray(w_out)
    out = np.empty((B, T, DIM), np.float32)
    for b in range(B):
        out[b] = results[2 * b]["y"] + results[2 * b + 1]["y"] + const[None, :]
    return out


# ---------------- self-contained kernel() entry point ----------------

_CACHED = {}


def _get_nc():
    if "nc" not in _CACHED:
        _CACHED["nc"] = build_kernel(T=2048, num_devices=8)
    return _CACHED["nc"]


def kernel(x, w_in, b_in, w_out, b_out):
    """Full-input MHA forward on 8 NeuronCores.

    x: (4, 2048, 768) f32; w_in: (768, 2304); b_in: (2304,);
    w_out: (768, 768); b_out: (768,). Returns (4, 2048, 768) f32.
    """
    from concourse.bass_utils import run_bass_kernel_spmd

    x = np.asarray(x, np.float32)
    w_in = np.asarray(w_in, np.float32)
    b_in = np.asarray(b_in, np.float32)
    w_out = np.asarray(w_out, np.float32)
    b_out = np.asarray(b_out, np.float32)

    nc = _get_nc()
    in_maps = host_prep(x, w_in, b_in, w_out, T=2048)
    res = run_bass_kernel_spmd(nc, in_maps, core_ids=list(range(8)))
    return host_post(res.results, b_out, b_in, w_out, B=4, T=2048)
